# revision 4
# baseline (speedup 1.0000x reference)
"""Trainium2 Bass kernel for nn_CrossVariateAdapter (sparse attention).

Full inputs -> full outputs. Shards batch B=16 over 8 NeuronCores (2 each).

Per batch:
  q,k projections: exact-ish via 4-term bf16 hi/lo split matmuls (fp32-grade).
  per-head scores S = q k^T via 4-term split (2x K=64-stacked matmul pairs).
  top-16 per row: DVE max8 -> match_replace -> max8 (threshold t16).
  softmax over top-16: U = exp(c*s) (ACT), mask = (s>=t16)*(1/denom) (GPSIMD),
  attn = U*mask (GPSIMD, bf16); attn transposed via DMA-xbar; attn@v on PE.
  A output: 3-term full-DM matmul of head-summed scores, same topk threshold.
"""

import numpy as np
import ml_dtypes

B, C, NP, DM, H, D, TOPK = 16, 1024, 512, 512, 8, 64, 16
NCORES = 8
BPC = B // NCORES  # batches per core
SCALE = float(D) ** -0.5
NEG_BIG = -3.0e38
USE_PE_TRANSPOSE = False

bf16 = ml_dtypes.bfloat16


def _split_bf16(x):
    hi = x.astype(bf16)
    lo = (x.astype(np.float32) - hi.astype(np.float32)).astype(bf16)
    return hi, lo


def _w_layout(w):
    # [NP_or_DM=512, 512] -> [128, 4, 512] with partition = row%128, chunk = row//128
    return np.ascontiguousarray(w.reshape(4, 128, 512).transpose(1, 0, 2))


def _build_program():
    import concourse.bass as bass
    import concourse.mybir as mybir
    import concourse.tile as tile
    from concourse import bacc

    dt = mybir.dt
    f32, bf = dt.float32, dt.bfloat16
    AF = mybir.ActivationFunctionType
    OP = mybir.AluOpType

    nc = bacc.Bacc(
        "TRN2",
        target_bir_lowering=False,
        debug=False,
        enable_asserts=False,
        num_devices=NCORES,
    )

    m_hi = nc.dram_tensor("m_hi", [BPC, C, NP], bf, kind="ExternalInput").ap()
    m_lo = nc.dram_tensor("m_lo", [BPC, C, NP], bf, kind="ExternalInput").ap()
    m_f = nc.dram_tensor("m_f32", [BPC, C, NP], f32, kind="ExternalInput").ap()
    wq_h = nc.dram_tensor("wq_h", [128, 4, 512], bf, kind="ExternalInput").ap()
    wq_l = nc.dram_tensor("wq_l", [128, 4, 512], bf, kind="ExternalInput").ap()
    wk_h = nc.dram_tensor("wk_h", [128, 4, 512], bf, kind="ExternalInput").ap()
    wk_l = nc.dram_tensor("wk_l", [128, 4, 512], bf, kind="ExternalInput").ap()
    wv_h = nc.dram_tensor("wv_h", [128, 4, 512], bf, kind="ExternalInput").ap()
    wo2_h = nc.dram_tensor("wo2_h", [128, 4, 512], bf, kind="ExternalInput").ap()
    bo2_hl = nc.dram_tensor("bo2_hl", [2, 512], bf, kind="ExternalInput").ap()
    mt_out = nc.dram_tensor("m_tilde", [BPC, C, NP], f32, kind="ExternalOutput").ap()
    a_out = nc.dram_tensor("a_mask", [BPC, C, C], f32, kind="ExternalOutput").ap()

    with tile.TileContext(nc) as tc:
        import contextlib

        ctx = contextlib.ExitStack()
        with ctx:
            p_w = ctx.enter_context(tc.tile_pool(name="w", bufs=1))
            p_mt = ctx.enter_context(tc.tile_pool(name="mt", bufs=1))
            p_qk = ctx.enter_context(tc.tile_pool(name="qk", bufs=1))
            p_v = ctx.enter_context(tc.tile_pool(name="v", bufs=1))
            p_ssb = ctx.enter_context(tc.tile_pool(name="ssb", bufs=3))
            p_tmp = ctx.enter_context(tc.tile_pool(name="tmp", bufs=2))
            p_u = ctx.enter_context(tc.tile_pool(name="u", bufs=3))
            p_msk = ctx.enter_context(tc.tile_pool(name="msk", bufs=2))
            p_at = ctx.enter_context(tc.tile_pool(name="at", bufs=3))
            p_att = ctx.enter_context(tc.tile_pool(name="att", bufs=10))
            p_sm = ctx.enter_context(tc.tile_pool(name="sm", bufs=6))
            p_io = ctx.enter_context(tc.tile_pool(name="io", bufs=2))
            ps_big = ctx.enter_context(tc.tile_pool(name="psb", bufs=2, space="PSUM"))
            ps_pv = ctx.enter_context(tc.tile_pool(name="pspv", bufs=2, space="PSUM"))
            ps_ot = ctx.enter_context(tc.tile_pool(name="psot", bufs=2, space="PSUM"))

            # persistent weights
            wqh = p_w.tile([128, 4, 512], bf, tag="wqh")
            wql = p_w.tile([128, 4, 512], bf, tag="wql")
            wkh = p_w.tile([128, 4, 512], bf, tag="wkh")
            wkl = p_w.tile([128, 4, 512], bf, tag="wkl")
            wvh = p_w.tile([128, 4, 512], bf, tag="wvh")
            wo2 = p_w.tile([128, 4, 512], bf, tag="wo2")
            bo2 = p_w.tile([2, 512], bf, tag="bo2")
            ones2 = p_w.tile([2, 128], bf, tag="ones2")
            for t, src in ((wqh, wq_h), (wql, wq_l), (wkh, wk_h), (wkl, wk_l),
                           (wvh, wv_h), (wo2, wo2_h), (bo2, bo2_hl)):
                nc.sync.dma_start(out=t[:], in_=src)
            nc.gpsimd.memset(ones2[:], 1.0)
            identity = None
            if USE_PE_TRANSPOSE:
                from concourse.masks import make_identity
                identity = p_w.tile([128, 128], bf, tag="ident")
                make_identity(nc, identity)

            for b in range(BPC):
                # ---- transposed M loads (bf16 hi/lo) ----
                mhiT = p_mt.tile([128, 4, 1024], bf, tag="mhiT")
                mloT = p_mt.tile([128, 4, 1024], bf, tag="mloT")
                for c in range(4):
                    nc.sync.dma_start(out=mhiT[:, c, :],
                                      in_=m_hi[b, :, c * 128:(c + 1) * 128],
                                      transpose=True)
                    nc.sync.dma_start(out=mloT[:, c, :],
                                      in_=m_lo[b, :, c * 128:(c + 1) * 128],
                                      transpose=True)

                # ---- q,k projections (4-term exact) ----
                qhi = p_qk.tile([128, 4, 1024], bf, tag="qhi")
                qlo = p_qk.tile([128, 4, 1024], bf, tag="qlo")
                khi = p_qk.tile([128, 4, 1024], bf, tag="khi")
                klo = p_qk.tile([128, 4, 1024], bf, tag="klo")
                for (wh, wl, thi, tlo) in ((wqh, wql, qhi, qlo), (wkh, wkl, khi, klo)):
                    for dmb in range(4):
                        for ih in range(2):
                            qp = ps_pv.tile([128, 512], f32, tag="pv")
                            n = 0
                            for mt_, wt in ((mhiT, wh), (mhiT, wl), (mloT, wh), (mloT, wl)):
                                for c in range(4):
                                    nc.tensor.matmul(
                                        qp[:],
                                        lhsT=wt[:, c, dmb * 128:(dmb + 1) * 128],
                                        rhs=mt_[:, c, ih * 512:(ih + 1) * 512],
                                        start=(n == 0), stop=(n == 15))
                                    n += 1
                            dst_h = thi[:, dmb, ih * 512:(ih + 1) * 512]
                            nc.scalar.activation(dst_h, qp[:], AF.Copy)
                            nc.vector.tensor_tensor(
                                out=tlo[:, dmb, ih * 512:(ih + 1) * 512],
                                in0=qp[:], in1=dst_h, op=OP.subtract)

                # ---- v projection (2-term: (Mh+Ml) @ Wv_hi) ----
                v_sb = p_v.tile([128, 8, 512], bf, tag="v")
                for jb in range(8):
                    vp = ps_pv.tile([128, 512], f32, tag="pv")
                    n = 0
                    for mt_ in (mhiT, mloT):
                        for c in range(4):
                            nc.tensor.matmul(
                                vp[:],
                                lhsT=mt_[:, c, jb * 128:(jb + 1) * 128],
                                rhs=wvh[:, c, :],
                                start=(n == 0), stop=(n == 7))
                            n += 1
                    nc.scalar.activation(v_sb[:, jb, :], vp[:], AF.Copy)

                # ---- output accumulator (transposed, bf16) ----
                outT = p_v.tile([128, 4, 1024], bf, tag="outT")

                # ---- per-head attention ----
                for h in range(8):
                    blk, r0 = h // 2, 64 * (h % 2)
                    attnT = [p_att.tile([128, 1024], bf, tag="attnT", name=f"attnT{j}")
                             for j in range(8)]
                    for ib in range(8):
                        sp = ps_big.tile([128, 1024], f32, tag="sc")
                        for jh in range(2):
                            n = 0
                            for lt, rt in ((qhi, khi), (qlo, klo), (qhi, klo), (qlo, khi)):
                                nc.tensor.matmul(
                                    sp[:, jh * 512:(jh + 1) * 512],
                                    lhsT=lt[r0:r0 + 64, blk, ib * 128:(ib + 1) * 128],
                                    rhs=rt[r0:r0 + 64, blk, jh * 512:(jh + 1) * 512],
                                    start=(n == 0), stop=(n == 3))
                                n += 1
                        s_sb = p_ssb.tile([128, 1024], f32, tag="ssb")
                        nc.scalar.activation(s_sb[:], sp[:], AF.Copy)
                        m8 = p_sm.tile([128, 16], f32, tag="m8")
                        nc.vector.max(out=m8[:, 0:8], in_=s_sb[:])
                        tmp = p_tmp.tile([128, 1024], f32, tag="tmp")
                        nc.vector.match_replace(out=tmp[:], in_to_replace=m8[:, 0:8],
                                                in_values=s_sb[:], imm_value=NEG_BIG)
                        nc.vector.max(out=m8[:, 8:16], in_=tmp[:])
                        e16 = p_sm.tile([128, 16], f32, tag="e16")
                        nc.scalar.activation(e16[:], m8[:], AF.Exp, scale=SCALE)
                        den = p_sm.tile([128, 1], f32, tag="den")
                        nc.vector.tensor_reduce(out=den[:], in_=e16[:],
                                                axis=mybir.AxisListType.X, op=OP.add)
                        rd = p_sm.tile([128, 1], f32, tag="rd")
                        nc.vector.reciprocal(rd[:], den[:])
                        u = p_u.tile([128, 1024], bf, tag="u")
                        nc.scalar.activation(u[:], s_sb[:], AF.Exp, scale=SCALE)
                        msk = p_msk.tile([128, 1024], f32, tag="msk")
                        nc.gpsimd.tensor_scalar(out=msk[:], in0=s_sb[:],
                                                scalar1=m8[:, 15:16], scalar2=rd[:],
                                                op0=OP.is_ge, op1=OP.mult)
                        at = p_at.tile([128, 1024], bf, tag="at")
                        nc.gpsimd.tensor_tensor(out=at[:], in0=u[:], in1=msk[:], op=OP.mult)
                        if USE_PE_TRANSPOSE:
                            for jb in range(8):
                                tp = ps_ot.tile([128, 128], bf, tag="tp")
                                nc.tensor.transpose(tp[:], at[:, jb * 128:(jb + 1) * 128],
                                                    identity[:])
                                nc.scalar.activation(
                                    attnT[jb][:, ib * 128:(ib + 1) * 128], tp[:], AF.Copy)
                        else:
                            for jb in range(8):
                                nc.sync.dma_start(
                                    out=attnT[jb][:, ib * 128:(ib + 1) * 128],
                                    in_=at[:, jb * 128:(jb + 1) * 128],
                                    transpose=True)
                    # attn @ v  (out transposed [d, i])
                    for ih in range(2):
                        op_ = ps_ot.tile([64, 512], f32, tag="ot")
                        for jc in range(8):
                            nc.tensor.matmul(
                                op_[:],
                                lhsT=v_sb[:, jc, h * 64:(h + 1) * 64],
                                rhs=attnT[jc][:, ih * 512:(ih + 1) * 512],
                                start=(jc == 0), stop=(jc == 7))
                        nc.scalar.activation(
                            outT[r0:r0 + 64, blk, ih * 512:(ih + 1) * 512],
                            op_[:], AF.Copy)

                # ---- A output: 3-term head-summed scores + topk mask ----
                for ib in range(8):
                    sa = ps_big.tile([128, 1024], f32, tag="sc")
                    for jh in range(2):
                        n = 0
                        for lt, rt in ((qhi, khi), (qlo, khi), (qhi, klo)):
                            for c in range(4):
                                nc.tensor.matmul(
                                    sa[:, jh * 512:(jh + 1) * 512],
                                    lhsT=lt[:, c, ib * 128:(ib + 1) * 128],
                                    rhs=rt[:, c, jh * 512:(jh + 1) * 512],
                                    start=(n == 0), stop=(n == 11))
                                n += 1
                    sa_sb = p_ssb.tile([128, 1024], f32, tag="ssb")
                    nc.scalar.activation(sa_sb[:], sa[:], AF.Copy)
                    m8a = p_sm.tile([128, 16], f32, tag="m8")
                    nc.vector.max(out=m8a[:, 0:8], in_=sa_sb[:])
                    tmpa = p_tmp.tile([128, 1024], f32, tag="tmp")
                    nc.vector.match_replace(out=tmpa[:], in_to_replace=m8a[:, 0:8],
                                            in_values=sa_sb[:], imm_value=NEG_BIG)
                    nc.vector.max(out=m8a[:, 8:16], in_=tmpa[:])
                    a01 = p_io.tile([128, 1024], f32, tag="a01")
                    nc.gpsimd.tensor_scalar(out=a01[:], in0=sa_sb[:],
                                            scalar1=m8a[:, 15:16], scalar2=None,
                                            op0=OP.is_ge)
                    nc.sync.dma_start(out=a_out[b, ib * 128:(ib + 1) * 128, :], in_=a01[:])

                # ---- delta projection + residual ----
                for ib in range(8):
                    dp = ps_pv.tile([128, 512], f32, tag="pv")
                    for c in range(4):
                        nc.tensor.matmul(dp[:], lhsT=outT[:, c, ib * 128:(ib + 1) * 128],
                                         rhs=wo2[:, c, :], start=(c == 0), stop=False)
                    nc.tensor.matmul(dp[:], lhsT=ones2[:], rhs=bo2[:],
                                     start=False, stop=True)
                    m_sb = p_io.tile([128, 512], f32, tag="msb")
                    nc.sync.dma_start(out=m_sb[:], in_=m_f[b, ib * 128:(ib + 1) * 128, :])
                    mt_sb = p_io.tile([128, 512], f32, tag="mts")
                    nc.vector.tensor_tensor(out=mt_sb[:], in0=dp[:], in1=m_sb[:], op=OP.add)
                    nc.sync.dma_start(out=mt_out[b, ib * 128:(ib + 1) * 128, :], in_=mt_sb[:])

    nc.finalize()
    return nc


def kernel(M, Wq, Wk, Wv, Wo, bo, gate):
    import concourse.bass_utils as bass_utils
    from concourse.bass_interp import get_hw_module

    M = np.asarray(M, np.float32)
    gate_f = np.float32(np.asarray(gate))

    Mhi, Mlo = _split_bf16(M)
    WqT = np.ascontiguousarray(np.asarray(Wq, np.float32).T)
    WkT = np.ascontiguousarray(np.asarray(Wk, np.float32).T)
    WvT = np.ascontiguousarray(np.asarray(Wv, np.float32).T)
    Wo2T = np.ascontiguousarray(gate_f * np.asarray(Wo, np.float32).T)
    bo2 = gate_f * np.asarray(bo, np.float32)
    wq_h, wq_l = _split_bf16(WqT)
    wk_h, wk_l = _split_bf16(WkT)
    wv_h, _ = _split_bf16(WvT)
    wo2_h, _ = _split_bf16(Wo2T)
    b_h, b_l = _split_bf16(bo2)
    bo2_hl = np.stack([b_h, b_l]).astype(bf16)

    nc = _build_program()
    nc.m = get_hw_module(nc.m)

    shared = {
        "wq_h": _w_layout(wq_h), "wq_l": _w_layout(wq_l),
        "wk_h": _w_layout(wk_h), "wk_l": _w_layout(wk_l),
        "wv_h": _w_layout(wv_h), "wo2_h": _w_layout(wo2_h),
        "bo2_hl": np.ascontiguousarray(bo2_hl),
    }
    in_maps = []
    for cix in range(NCORES):
        sl = slice(cix * BPC, (cix + 1) * BPC)
        in_maps.append({
            "m_hi": np.ascontiguousarray(Mhi[sl]),
            "m_lo": np.ascontiguousarray(Mlo[sl]),
            "m_f32": np.ascontiguousarray(M[sl]),
            **shared,
        })

    res = bass_utils.run_bass_kernel_spmd(nc, in_maps, core_ids=list(range(NCORES)))
    global LAST_RESULTS
    LAST_RESULTS = res
    mt = np.concatenate([r["m_tilde"] for r in res.results], axis=0)
    am = np.concatenate([r["a_mask"] for r in res.results], axis=0)
    return mt.astype(np.float32), am.astype(np.float32)


if __name__ == "__main__":
    pass
